# revision 20
# baseline (speedup 1.0000x reference)
"""Trainium2 Bass kernel for a full attention block (B=4, S=2048, H=1024, 16 heads).

Sharding: 8 cores = (batch b = core//2) x (query-half qh = core%2).
Each core computes the complete pipeline for its 1024 query rows of batch b:
QKV projections, 16-head attention over all 2048 keys, output projection,
residual add and layernorm.  No cross-core communication; the host slices
inputs and concatenates the 8 disjoint output shards.

v2 (fp8 DoubleRow + dual-engine softmax), tuned from the v1 trace where the
PE was the bottleneck (397us busy of 453us) and ACT second (283us):
  - All four projections and the probs@V / ctx@Wo matmuls run as fp8e4
    DoubleRow matmuls (two 128-deep k-tiles per instruction, 2x PE rate).
    Weights are pre-scaled x4 so their sigma ~ 0.124 sits in fp8e4's normal
    range; the compensation folds into the softmax exp scale (1/128 = 1/8
    attention scale * 1/16 for Wq,Wk x4) and the residual add (1/16 for
    Wv,Wo x4).  Scores (contraction = head dim 64, un-pairable) stay bf16.
  - Softmax probs are fp8e5 (e5m2): wide exponent range holds exp() of any
    plausible score.  The ACT engine computes most tiles (Exp activation,
    scale=1/128, fp8 output); a tunable subset runs on the otherwise lighter
    DVE as a Schraudolph exp: i8 = round(a*score + b) then bitcast e5m2,
    which is exact in the exponent and ~+-9% on the mantissa interpolation -
    the same order as e5m2 quantization itself.
  - ctx accumulates unnormalized with a ones column in V giving the softmax
    denominator L as PSUM row 64; 1/L via one DVE reciprocal_approx_fast
    straight from PSUM, replicated by a Pool partition_broadcast, and the
    normalization multiply is fused into the PSUM->SBUF drain
    (scalar_tensor_tensor), writing fp8 CTX for the DoubleRow out-proj.
  - Pool (which cannot touch PSUM on TRN2, and runs tensor ops at a tiny
    fraction of roofline) only does the 1/L partition broadcasts + memsets.
"""

import numpy as np
import ml_dtypes

B, S, H, NH, DH = 4, 2048, 1024, 16, 64
P = 128
NCORES = 8
SQ = 1024        # query rows per core
EPS = 1e-12

W_SCALE = 4.0                       # weight pre-scale into fp8e4 range
EXP_SCALE = 0.125 / (W_SCALE * W_SCALE)   # 1/sqrt(DH) / (Wq*Wk scales)
OUT_SCALE = 1.0 / (W_SCALE * W_SCALE)     # undo Wv*Wo scales after out-proj
LOG2E = 1.4426950408889634
# Schraudolph exp -> e5m2: i8 = sp * EXP_A + EXP_B, bitcast to fp8e5.
# value = 2^((i-60)/4); a = 4*EXP_SCALE*log2(e); b centers the sawtooth
# (-0.17) and compensates truncation (+0.5 if the convert truncates).
EXP_A = 4.0 * EXP_SCALE * LOG2E
EXP_B = 60.0 - 0.172
DVE_KTS = (4, 5, 10, 11)            # score k-tiles whose exp runs on DVE

_CACHE = {}


def _build_program(use_bias, use_affine):
    from concourse import bacc, tile, mybir

    f32 = mybir.dt.float32
    bf16 = mybir.dt.bfloat16
    fp8 = mybir.dt.float8e4
    fp8e5 = mybir.dt.float8e5
    i8 = mybir.dt.int8
    AF = mybir.ActivationFunctionType
    OP = mybir.AluOpType
    DR = mybir.MatmulPerfMode.DoubleRow

    HP = H + P if use_bias else H   # padded contraction (bias ones row)
    KO = HP // P                    # projection contraction subtiles
    KC = H // P + (1 if use_bias else 0)  # out-proj contraction subtiles

    nc = bacc.Bacc("TRN2", target_bir_lowering=False, debug=False,
                   num_devices=NCORES)

    xT_d = nc.dram_tensor("xT", [HP, S], fp8, kind="ExternalInput").ap()
    xq_d = nc.dram_tensor("xq", [SQ, H], f32, kind="ExternalInput").ap()
    wqT_d = nc.dram_tensor("wqT", [HP, H], fp8, kind="ExternalInput").ap()
    wkT_d = nc.dram_tensor("wkT", [HP, H], fp8, kind="ExternalInput").ap()
    wvT_d = nc.dram_tensor("wvT", [HP, H], fp8, kind="ExternalInput").ap()
    woT_d = nc.dram_tensor("woT", [P * KC, H], fp8, kind="ExternalInput").ap()
    if use_affine:
        gam_d = nc.dram_tensor("gam", [P, H], f32, kind="ExternalInput").ap()
        bet_d = nc.dram_tensor("bet", [P, H], f32, kind="ExternalInput").ap()
    out_d = nc.dram_tensor("out", [SQ, H], f32, kind="ExternalOutput").ap()

    xT_r = xT_d.rearrange("(o p) s -> p o s", p=P)      # [128, KO, 2048]
    wqT_r = wqT_d.rearrange("(o p) j -> p o j", p=P)
    wkT_r = wkT_d.rearrange("(o p) j -> p o j", p=P)
    wvT_r = wvT_d.rearrange("(o p) j -> p o j", p=P)
    woT_r = woT_d.rearrange("(o p) j -> p o j", p=P)
    xq_r = xq_d.rearrange("(t p) j -> p t j", p=P)      # [128, 8, 1024]
    out_r = out_d.rearrange("(t p) j -> p t j", p=P)

    with tile.TileContext(nc) as tc:
        with tc.tile_pool(name="pers", bufs=1) as pers, \
             tc.tile_pool(name="probs", bufs=4) as probs_pool, \
             tc.tile_pool(name="att1", bufs=2) as att1, \
             tc.tile_pool(name="wop", bufs=1) as wop:
            KT = pers.tile([P, 8, S], fp8)       # [p, jt, s]; j = jt*128+p
            QT = pers.tile([P, 8, SQ], fp8)
            V = pers.tile([P, 16, NH, 66], fp8)   # [k%128, k//128, head, d|1@64]
            CTX = pers.tile([P, KC, SQ], fp8)     # ctxT (+ ones row subtile)

            nc.gpsimd.memset(V[:, :, :, 64:65], 1.0)
            if use_bias:
                nc.gpsimd.memset(CTX[:, KC - 1, :], 0.0)
                # ones value W_SCALE so bias row (x4) reaches 16*bo
                nc.gpsimd.memset(CTX[0:1, KC - 1, :], W_SCALE)

            with tc.tile_pool(name="spsum", bufs=2, space="PSUM") as spsum, \
                 tc.tile_pool(name="ppsum", bufs=2, space="PSUM") as ppsum, \
                 tc.tile_pool(name="cpsum", bufs=2, space="PSUM") as cpsum:

                def emit_pair(pair):
                    hA, hB = 2 * pair, 2 * pair + 1
                    jt = pair
                    # two q-chunk halves; each [128,1024] score psum tile
                    # holds both heads (cols 0:512 head A, 512: head B) so a
                    # single exp op serves the pair; probs tiles pack kt
                    # PAIRS ([p, 2, 1024]) for the DoubleRow ctx matmuls.
                    for c in range(2):
                        cs = slice(c * 512, (c + 1) * 512)
                        ctxps = {h: cpsum.tile([65, 512], f32, tag="ctx",
                                               name=f"ctxps_{h}_{c}")
                                 for h in (hA, hB)}
                        prs = {}
                        for kt in range(16):
                            t2, j2 = kt // 2, kt % 2
                            if j2 == 0:
                                prs[t2] = probs_pool.tile(
                                    [P, 2, SQ], fp8e5, tag="pt",
                                    name=f"pr_{t2}_{c}")
                            pr = prs[t2]
                            sp = spsum.tile([P, SQ], f32, tag="sp",
                                            name=f"sps_{kt}_{c}")
                            for h in (hA, hB):
                                base = (h % 2) * 64
                                nc.tensor.matmul(
                                    sp[:, base * 8:base * 8 + 512],
                                    KT[base:base + 64, jt, kt * P:(kt + 1) * P],
                                    QT[base:base + 64, jt, cs],
                                    start=True, stop=True)
                            if kt in DVE_KTS:
                                nc.vector.tensor_scalar(
                                    pr[:, j2, :].bitcast(i8), sp[:],
                                    float(EXP_A), float(EXP_B),
                                    OP.mult, OP.add)
                            else:
                                nc.scalar.activation(pr[:, j2, :], sp[:],
                                                     AF.Exp,
                                                     scale=float(EXP_SCALE))
                            if j2 == 1:
                                for h in (hA, hB):
                                    base = (h % 2) * 64
                                    nc.tensor.matmul(
                                        ctxps[h][:],
                                        V[:, 2 * t2:2 * t2 + 2, h, 0:65],
                                        pr[:, 0:2, base * 8:base * 8 + 512],
                                        start=(t2 == 0), stop=(t2 == 7),
                                        perf_mode=DR)
                        # denominator L = psum row 64 -> SBUF, bounce to
                        # partition 0 (reciprocal_approx_fast misreads PSUM
                        # on hw), 1/L, Pool-broadcast, and fold the multiply
                        # into the PSUM->SBUF fp8 drain.
                        for h in (hA, hB):
                            base = (h % 2) * 64
                            lstage = att1.tile([65, 512], f32, tag="lstage",
                                               name=f"lstage_{h}_{c}")
                            nc.vector.tensor_copy(lstage[64:65, :],
                                                  ctxps[h][64:65, :])
                            lr0 = att1.tile([1, 512], f32, tag="lr0",
                                            name=f"lr0_{h}_{c}")
                            nc.sync.dma_start(lr0[:], lstage[64:65, :])
                            lrec = att1.tile([1, 512], f32, tag="lrec",
                                             name=f"lrec_{h}_{c}")
                            nc.vector.reciprocal_approx_fast(lrec[:],
                                                             lr0[:])
                            lrep = att1.tile([P, 512], f32, tag="lrep",
                                             name=f"lrep_{h}_{c}")
                            nc.gpsimd.partition_broadcast(lrep[:], lrec[0:1, :])
                            if base == 0:
                                nc.vector.scalar_tensor_tensor(
                                    CTX[0:64, jt, cs], ctxps[h][0:64, :],
                                    1.0, lrep[0:64, :], OP.bypass, OP.mult)
                            else:
                                cstage = att1.tile([64, 512], fp8,
                                                   tag="cstage",
                                                   name=f"cstage_{h}_{c}")
                                nc.vector.scalar_tensor_tensor(
                                    cstage[:], ctxps[h][0:64, :],
                                    1.0, lrep[0:64, :], OP.bypass, OP.mult)
                                nc.sync.dma_start(CTX[64:128, jt, cs],
                                                  cstage[:])

                with tc.tile_pool(name="proj", bufs=1) as projp, \
                     tc.tile_pool(name="wstr", bufs=2) as wstr, \
                     tc.tile_pool(name="wvstr", bufs=1) as wvstr:
                    XT = projp.tile([P, KO, S], fp8)
                    # issue the first Q-weight tile ahead of the bulk XT
                    # chunks so the PE can start at chunk 1, not chunk 8
                    wq0_t = wstr.tile([P, KO, P], fp8, tag="w", name="wq_0")
                    nc.sync.dma_start(wq0_t[:], wqT_r[:, :, 0:P])
                    # per-subtile loads so the first projection matmul can
                    # start as soon as chunk 0 lands
                    for ko in range(KO):
                        nc.sync.dma_start(XT[:, ko, :], xT_r[:, ko, :])

                    def proj_mm(ps, w_t, cs):
                        for k2 in range(KO // 2):
                            nc.tensor.matmul(
                                ps[:], w_t[:, 2 * k2:2 * k2 + 2, :],
                                XT[:, 2 * k2:2 * k2 + 2, cs],
                                start=(k2 == 0),
                                stop=(KO % 2 == 0 and k2 == KO // 2 - 1),
                                perf_mode=DR)
                        if KO % 2:
                            nc.tensor.matmul(
                                ps[:], w_t[:, KO - 1, :], XT[:, KO - 1, cs],
                                start=False, stop=True)

                    def emit_qk(jt, wq_t=None):
                        if wq_t is None:
                            wq_t = wstr.tile([P, KO, P], fp8, tag="w",
                                             name=f"wq_{jt}")
                            nc.sync.dma_start(wq_t[:],
                                              wqT_r[:, :, jt * P:(jt + 1) * P])
                        for sc in range(2):
                            ps = ppsum.tile([P, 512], f32, tag="pp",
                                            name=f"qps_{jt}_{sc}")
                            proj_mm(ps, wq_t, slice(sc * 512, sc * 512 + 512))
                            nc.vector.tensor_copy(
                                QT[:, jt, sc * 512:(sc + 1) * 512], ps[:])
                        wk_t = wstr.tile([P, KO, P], fp8, tag="w",
                                         name=f"wk_{jt}")
                        nc.sync.dma_start(wk_t[:],
                                          wkT_r[:, :, jt * P:(jt + 1) * P])
                        for sc in range(4):
                            ps = ppsum.tile([P, 512], f32, tag="pp",
                                            name=f"kps_{jt}_{sc}")
                            proj_mm(ps, wk_t, slice(sc * 512, sc * 512 + 512))
                            nc.vector.tensor_copy(
                                KT[:, jt, sc * 512:(sc + 1) * 512], ps[:])

                    def emit_v(jc):
                        wv_t = wvstr.tile([P, KO, 512], fp8, tag="wv",
                                          name=f"wv_{jc}")
                        nc.sync.dma_start(
                            wv_t[:], wvT_r[:, :, jc * 512:(jc + 1) * 512])
                        for st in range(16):
                            ps = ppsum.tile([P, 512], f32, tag="pp",
                                            name=f"vps_{st}_{jc}")
                            for k2 in range(KO // 2):
                                nc.tensor.matmul(
                                    ps[:],
                                    XT[:, 2 * k2:2 * k2 + 2, st * P:(st + 1) * P],
                                    wv_t[:, 2 * k2:2 * k2 + 2, :],
                                    start=(k2 == 0),
                                    stop=(KO % 2 == 0 and k2 == KO // 2 - 1),
                                    perf_mode=DR)
                            if KO % 2:
                                nc.tensor.matmul(
                                    ps[:], XT[:, KO - 1, st * P:(st + 1) * P],
                                    wv_t[:, KO - 1, :],
                                    start=False, stop=True)
                            nc.vector.tensor_copy(
                                V[:, st, jc * 8:(jc + 1) * 8, 0:64],
                                ps[:].rearrange("p (h d) -> p h d", d=64))

                    # V's second half feeds only pairs 4-7: emit it late so
                    # it acts as PE filler once the QK stream runs dry
                    emit_qk(0, wq_t=wq0_t)
                    emit_v(0)
                    emit_pair(0)
                    for jt in (1, 2, 3):
                        emit_qk(jt)
                        emit_pair(jt)
                    emit_v(1)
                    for jt in (4, 5, 6):
                        emit_qk(jt)
                        emit_pair(jt)
                    emit_qk(7)

                # XT freed; stream the output-projection weight during the
                # last attention pair
                WO = wop.tile([P, KC, H], fp8)
                nc.sync.dma_start(WO[:], woT_r[:])
                emit_pair(7)

                # ---- output projection + layernorm: emitted inside the
                # ---- attention PSUM scope (hp reuses the projection tag) so
                # ---- the scheduler can hoist matmuls into pair 7's
                # ---- ACT-bound stretch as PE filler
                with tc.tile_pool(name="epi2", bufs=3) as epi:
                    if use_affine:
                        GAM = epi.tile([P, H], f32, tag="gam")
                        BET = epi.tile([P, H], f32, tag="bet")
                        nc.sync.dma_start(GAM[:], gam_d[:])
                        nc.sync.dma_start(BET[:], bet_d[:])
                    for qt in range(8):
                        xqt = epi.tile([P, H], f32, tag="xq")
                        nc.sync.dma_start(xqt[:], xq_r[:, qt, :])
                        tmp = epi.tile([P, H], f32, tag="tmp")
                        for jc in range(2):
                            hp = ppsum.tile([P, 512], f32, tag="pp",
                                            name=f"hps_{qt}_{jc}")
                            for k2 in range(KC // 2):
                                nc.tensor.matmul(
                                    hp[:],
                                    CTX[:, 2 * k2:2 * k2 + 2, qt * P:(qt + 1) * P],
                                    WO[:, 2 * k2:2 * k2 + 2, jc * 512:(jc + 1) * 512],
                                    start=(k2 == 0),
                                    stop=(KC % 2 == 0 and k2 == KC // 2 - 1),
                                    perf_mode=DR)
                            if KC % 2:
                                nc.tensor.matmul(
                                    hp[:], CTX[:, KC - 1, qt * P:(qt + 1) * P],
                                    WO[:, KC - 1, jc * 512:(jc + 1) * 512],
                                    start=False, stop=True)
                            nc.vector.scalar_tensor_tensor(
                                tmp[:, jc * 512:(jc + 1) * 512], hp[:],
                                float(OUT_SCALE),
                                xqt[:, jc * 512:(jc + 1) * 512],
                                OP.mult, OP.add)
                        stats = epi.tile([P, 2, 6], f32, tag="st")
                        mv = epi.tile([P, 2], f32, tag="mv")
                        for c in range(2):
                            nc.vector.bn_stats(
                                stats[:, c, :], tmp[:, c * 512:(c + 1) * 512])
                        nc.vector.bn_aggr(mv[:], stats[:])
                        ve = epi.tile([P, 1], f32, tag="ve")
                        nc.vector.tensor_scalar_add(ve[:], mv[:, 1:2], float(EPS))
                        sd = epi.tile([P, 1], f32, tag="sd")
                        nc.scalar.activation(sd[:], ve[:], AF.Sqrt)
                        rstd = epi.tile([P, 1], f32, tag="rstd")
                        nc.vector.reciprocal(rstd[:], sd[:])
                        osb = epi.tile([P, H], f32, tag="osb")
                        nc.vector.tensor_scalar(
                            osb[:], tmp[:], mv[:, 0:1], rstd[:],
                            OP.subtract, OP.mult)
                        if use_affine:
                            nc.vector.tensor_tensor(osb[:], osb[:], GAM[:],
                                                    OP.mult)
                            nc.vector.tensor_tensor(osb[:], osb[:], BET[:],
                                                    OP.add)
                        nc.sync.dma_start(out_r[:, qt, :], osb[:])

    nc.compile()
    return nc


def _get_program(use_bias, use_affine):
    key = (use_bias, use_affine)
    if key not in _CACHE:
        _CACHE[key] = _build_program(use_bias, use_affine)
    return _CACHE[key]


def _prep_inputs(input_tensor, Wq, bq, Wk, bk, Wv, bv, Wo, bo, gamma, beta,
                 use_bias, use_affine):
    f8 = ml_dtypes.float8_e4m3
    x = np.asarray(input_tensor, np.float32)
    HP = H + P if use_bias else H

    def padw(w, b):
        m = np.zeros((HP, H), np.float32)
        m[:H] = np.asarray(w, np.float32).T * W_SCALE
        if use_bias:
            m[H] = np.asarray(b, np.float32) * W_SCALE
        return m.astype(f8)

    wqT = padw(Wq, bq)
    wkT = padw(Wk, bk)
    wvT = padw(Wv, bv)
    woT = padw(Wo, bo)

    in_maps = []
    for core in range(NCORES):
        b, qh = core // 2, core % 2
        xb = x[b]
        rolled = np.concatenate(
            [xb[qh * SQ:(qh + 1) * SQ], xb[(1 - qh) * SQ:(2 - qh) * SQ]], 0)
        xT = np.zeros((HP, S), np.float32)
        xT[:H] = rolled.T
        if use_bias:
            xT[H] = 1.0
        m = {
            "xT": xT.astype(f8),
            "xq": np.ascontiguousarray(xb[qh * SQ:(qh + 1) * SQ]),
            "wqT": wqT, "wkT": wkT, "wvT": wvT, "woT": woT,
        }
        if use_affine:
            m["gam"] = np.ascontiguousarray(np.broadcast_to(
                np.asarray(gamma, np.float32), (P, H)))
            m["bet"] = np.ascontiguousarray(np.broadcast_to(
                np.asarray(beta, np.float32), (P, H)))
        in_maps.append(m)
    return in_maps


def run(inputs, trace=False, tmpdir=None):
    from concourse.bass_utils import run_bass_kernel_spmd
    use_bias = any(
        np.any(np.asarray(inputs[k], np.float32) != 0.0)
        for k in ("bq", "bk", "bv", "bo"))
    use_affine = bool(
        np.any(np.asarray(inputs["gamma"], np.float32) != 1.0)
        or np.any(np.asarray(inputs["beta"], np.float32) != 0.0))
    nc = _get_program(use_bias, use_affine)
    in_maps = _prep_inputs(use_bias=use_bias, use_affine=use_affine, **inputs)
    res = run_bass_kernel_spmd(nc, in_maps, list(range(NCORES)), trace=trace,
                               tmpdir=tmpdir)
    out = np.zeros((B, S, H), np.float32)
    for core in range(NCORES):
        b, qh = core // 2, core % 2
        out[b, qh * SQ:(qh + 1) * SQ] = res.results[core]["out"]
    return out, res


def kernel(**inputs):
    out, _ = run(inputs, trace=False)
    return out
